# revision 35
# baseline (speedup 1.0000x reference)
"""GCNConv mean-aggregation kernel for 8 Trainium2 NeuronCores.

Reference computation:
    msgs   = x[src]                       # [E, D] gather
    summed = segment_sum(msgs, dst, N)    # [N, D]
    deg    = segment_sum(ones, dst, N)    # [N]
    h      = summed / max(deg, 1)
    out    = h @ W.T + b                  # [N, D_OUT]

Strategy (no collectives needed):
  - Shard edges by dst node ownership: core c owns a contiguous slice of
    6272 nodes.  Within a core, nodes are re-packed into 98 windows of 64
    via a 2D greedy balance so every window has nearly equal lo/hi edge
    counts (minimizes cross-core SPMD padding).
  - Gather tables hold x in bf16, 256B rows (64 feats + 64 zero pad);
    indices are int16, so x is split at src=32767 into lo/hi tables,
    each with a zero row at index 0 used by padding lanes.
  - Edges form two CONTINUOUS per-class streams (lo/hi), window-major,
    with no per-window 128-alignment: a 128-edge subtile may straddle two
    windows.  Each window processes its subtile range twice-shared
    boundaries included; out-of-window lanes carry dst_rel=-1 which never
    matches the iota, so they contribute zero to that window's matmul.
  - dma_gather calls cover CHUNK=8 subtiles (1024 indices) with
    single_packet coalescing, rotated over the 4 SWDGE queues.  1024 is
    the hard per-call limit (64 descriptors/lane; 1152 wedges the SDMA
    engine).  prepare_only+trigger_dma was tried and abandoned: tile's
    pass-2 emits no consumer RAW waits for gen_mode=1 gathers and its
    scheduler separates triggers from preps, which misfires the ring and
    crashes the device even with manual gating.
  - Aggregation per window: [128e, 64n] one-hot from dst via DVE is_equal
    (bf16, built in 4-window batches), then msgs.T @ onehot accumulated
    into a [64, 4*64] PSUM group tile = h.T directly (no transpose pass).
  - Degree reciprocals are host-computed, shipped replicated [64, NPC] in
    window-position order; normalization is one DVE multiply per 4-window
    group writing bf16 h.T slices.
  - Dense layer: z = W @ h.T per 512-column tile (bf16), bias on DVE,
    out.T written per tile.  Host scatters columns back to node order.
"""

import sys

sys.path.insert(0, "/opt/trn_rl_repo")

import ml_dtypes
import numpy as np

import concourse.bacc as bacc
import concourse.mybir as mybir
import concourse.tile as tile
from concourse.bass_utils import run_bass_kernel_spmd

N_NODES = 50000
N_EDGES = 800000
D = 64
N_CORES = 8
NPC = 6272          # nodes per core (= 98 windows of 64)
WIN = 64            # dst-window width per PSUM accumulation group
N_WIN = NPC // WIN  # 98
SPLIT = 32767       # src < SPLIT -> lo table, else hi table
ROW = 128           # gather row: 64 bf16 feats + 64 bf16 zero pad (256 B)
CHUNK = 8           # subtiles (of 128 edges) per dma_gather call
NQ = 4              # SWDGE queues for parallel gather descriptor work
ZCOL = 512          # output columns per dense-layer matmul tile
GRP = 4             # windows per one-hot/normalize batch
PREP = False        # prepare_only + trigger_dma crashes the device (ucode
                    # misfires when tile reorders triggers); keep gen_mode=0

F32 = mybir.dt.float32
BF16 = mybir.dt.bfloat16
I16 = mybir.dt.int16

BF = ml_dtypes.bfloat16

# Results of the most recent run (for test harness inspection).
LAST = {}


def _balance_core(a, b):
    """Greedy 2D balance of nodes (edge-count vectors a, b) into N_WIN
    bins of <=WIN slots.  Returns (win_of, slot_of) per node."""
    n = len(a)
    tA = max(a.sum() / N_WIN, 1.0)
    tB = max(b.sum() / N_WIN, 1.0)
    binA = np.zeros(N_WIN)
    binB = np.zeros(N_WIN)
    binN = np.zeros(N_WIN, dtype=np.int64)
    win_of = np.zeros(n, dtype=np.int64)
    slot_of = np.zeros(n, dtype=np.int64)
    order = np.argsort(-(a + b), kind="stable")
    for i in order:
        score = (binA + a[i]) / tA + (binB + b[i]) / tB
        score[binN >= WIN] = np.inf
        w = int(np.argmin(score))
        win_of[i] = w
        slot_of[i] = binN[w]
        binN[w] += 1
        binA[w] += a[i]
        binB[w] += b[i]
    return win_of, slot_of


def _prep(x, src, dst):
    """Host-side sharding. Returns gather tables, per-core streams and
    the shared (SPMD) program geometry."""
    x = np.asarray(x, dtype=np.float32)
    src = np.asarray(src, dtype=np.int64)
    dst = np.asarray(dst, dtype=np.int64)

    xlo = np.zeros((SPLIT + 1, ROW), dtype=BF)
    xlo[1:, :D] = x[:SPLIT].astype(BF)
    xhi = np.zeros((N_NODES - SPLIT + 1, ROW), dtype=BF)
    xhi[1:, :D] = x[SPLIT:].astype(BF)

    cls = (src >= SPLIT).astype(np.int64)
    degA = np.bincount(dst[cls == 0], minlength=N_NODES).astype(np.int64)
    degB = np.bincount(dst[cls == 1], minlength=N_NODES).astype(np.int64)
    deg = (degA + degB).astype(np.float32)
    rec = 1.0 / np.maximum(deg, 1.0)

    # cross-core balance: assign nodes to cores so every core carries a
    # nearly equal lo/hi edge load (the SPMD program is sized by the
    # heaviest core), then pack windows within each core.
    tA8 = max(degA.sum() / N_CORES, 1.0)
    tB8 = max(degB.sum() / N_CORES, 1.0)
    coreA = np.zeros(N_CORES)
    coreB = np.zeros(N_CORES)
    coreN = np.zeros(N_CORES, dtype=np.int64)
    core_assign = np.zeros(N_NODES, dtype=np.int64)
    for n in np.argsort(-(degA + degB), kind="stable"):
        score = (coreA + degA[n]) / tA8 + (coreB + degB[n]) / tB8
        score[coreN >= NPC] = np.inf
        c = int(np.argmin(score))
        core_assign[n] = c
        coreN[c] += 1
        coreA[c] += degA[n]
        coreB[c] += degB[n]
    nodes_of_core = [np.where(core_assign == c)[0] for c in range(N_CORES)]
    local_of = np.zeros(N_NODES, dtype=np.int64)
    for c in range(N_CORES):
        local_of[nodes_of_core[c]] = np.arange(len(nodes_of_core[c]))

    core_of = core_assign[dst]  # [E]

    # per-core window packing
    win_of = [None] * N_CORES   # node-local idx -> window
    slot_of = [None] * N_CORES
    perm = [None] * N_CORES     # position p -> window id
    cntA = np.zeros((N_CORES, N_WIN), dtype=np.int64)
    cntB = np.zeros((N_CORES, N_WIN), dtype=np.int64)
    for c in range(N_CORES):
        a = degA[nodes_of_core[c]].astype(np.float64)
        b = degB[nodes_of_core[c]].astype(np.float64)
        w_of, s_of = _balance_core(a, b)
        win_of[c], slot_of[c] = w_of, s_of
        cA = np.bincount(w_of, weights=a, minlength=N_WIN).astype(np.int64)
        cB = np.bincount(w_of, weights=b, minlength=N_WIN).astype(np.int64)
        p = np.argsort(-cA, kind="stable")
        perm[c] = p
        cntA[c] = cA[p]
        cntB[c] = cB[p]

    capA = cntA.max(axis=0)  # [N_WIN] per-position capacity, lo stream
    capB = cntB.max(axis=0)
    PA = np.zeros(N_WIN + 1, dtype=np.int64)
    np.cumsum(capA, out=PA[1:])
    PB = np.zeros(N_WIN + 1, dtype=np.int64)
    np.cumsum(capB, out=PB[1:])
    SA_sub = int(-(-PA[-1] // 128))
    SB_sub = int(-(-PB[-1] // 128))

    # view geometry (shared across cores)
    vs0A = (PA[:-1] // 128).astype(np.int64)
    vs1A = np.minimum(-(-(PA[:-1] + np.maximum(capA, 1)) // 128), SA_sub)
    vs0B = (PB[:-1] // 128).astype(np.int64)
    vs1B = np.minimum(-(-(PB[:-1] + np.maximum(capB, 1)) // 128), SB_sub)
    nvA = vs1A - vs0A
    nvB = vs1B - vs0B
    pvA = np.zeros(N_WIN + 1, dtype=np.int64)
    np.cumsum(nvA, out=pvA[1:])
    pvB = np.zeros(N_WIN + 1, dtype=np.int64)
    np.cumsum(nvB, out=pvB[1:])
    NVA = int(pvA[-1])
    NVB = int(pvB[-1])
    MAXV = int(max(nvA.max(), nvB.max()))

    geom = dict(
        capA=capA, capB=capB, PA=PA, PB=PB, SA=SA_sub, SB=SB_sub,
        vs0A=vs0A, vs1A=vs1A, vs0B=vs0B, vs1B=vs1B,
        pvA=pvA, pvB=pvB, NVA=NVA, NVB=NVB, MAXV=MAXV,
    )

    # per-core edge streams
    # group edges by (core, class, window-position)
    pos_of_win = np.zeros((N_CORES, N_WIN), dtype=np.int64)
    for c in range(N_CORES):
        pos_of_win[c][perm[c]] = np.arange(N_WIN)
    nl = local_of[dst]  # node-local id
    wo = np.zeros(N_EDGES, dtype=np.int64)
    so = np.zeros(N_EDGES, dtype=np.int64)
    for c in range(N_CORES):
        m = core_of == c
        wo[m] = win_of[c][nl[m]]
        so[m] = slot_of[c][nl[m]]
    po = pos_of_win[core_of, wo]  # position of each edge's window
    key = ((core_of * 2 + cls) * N_WIN + po)
    order = np.argsort(key, kind="stable")
    src_s, so_s, key_s = src[order], so[order], key[order]
    counts = np.bincount(key_s, minlength=N_CORES * 2 * N_WIN)
    starts = np.zeros(N_CORES * 2 * N_WIN + 1, dtype=np.int64)
    np.cumsum(counts, out=starts[1:])

    per_core = []
    for c in range(N_CORES):
        iA = np.zeros(SA_sub * 128, dtype=np.int16)
        dA = np.full(SA_sub * 128, -1.0, dtype=np.float32)
        iB = np.zeros(max(SB_sub, 1) * 128, dtype=np.int16)
        dB = np.full(max(SB_sub, 1) * 128, -1.0, dtype=np.float32)
        for p in range(N_WIN):
            g = (c * 2 + 0) * N_WIN + p
            s0, s1 = starts[g], starts[g + 1]
            q0 = int(PA[p])
            iA[q0 : q0 + (s1 - s0)] = (src_s[s0:s1] + 1).astype(np.int16)
            dA[q0 : q0 + (s1 - s0)] = so_s[s0:s1].astype(np.float32)
            g = (c * 2 + 1) * N_WIN + p
            s0, s1 = starts[g], starts[g + 1]
            q0 = int(PB[p])
            iB[q0 : q0 + (s1 - s0)] = (src_s[s0:s1] - SPLIT + 1).astype(
                np.int16
            )
            dB[q0 : q0 + (s1 - s0)] = so_s[s0:s1].astype(np.float32)

        # view dst streams: mask out-of-position lanes to -1
        dvA = np.full((NVA, 128), -1.0, dtype=np.float32)
        for p in range(N_WIN):
            lo, hi = int(PA[p]), int(PA[p] + capA[p])
            for j, s in enumerate(range(int(vs0A[p]), int(vs1A[p]))):
                g0 = s * 128
                col = dvA[int(pvA[p]) + j]
                lanes = np.arange(g0, g0 + 128)
                m = (lanes >= lo) & (lanes < hi)
                col[m] = dA[lanes[m]]
        dvB = np.full((max(NVB, 1), 128), -1.0, dtype=np.float32)
        for p in range(N_WIN):
            lo, hi = int(PB[p]), int(PB[p] + capB[p])
            for j, s in enumerate(range(int(vs0B[p]), int(vs1B[p]))):
                g0 = s * 128
                col = dvB[int(pvB[p]) + j]
                lanes = np.arange(g0, g0 + 128)
                m = (lanes >= lo) & (lanes < hi)
                col[m] = dB[lanes[m]]

        # node order (position-major) for rec + output mapping
        node_pos = np.full(NPC, -1, dtype=np.int64)
        locs = pos_of_win[c][win_of[c]] * WIN + slot_of[c]
        node_pos[locs] = nodes_of_core[c]
        rec_cols = np.ones(NPC, dtype=np.float32)
        valid = node_pos >= 0
        rec_cols[valid] = rec[node_pos[valid]]
        rec_c = np.ascontiguousarray(
            np.tile(rec_cols[None, :], (D, 1))
        )

        per_core.append(
            dict(
                iA=iA, iB=iB,
                dvA=np.ascontiguousarray(dvA.T),
                dvB=np.ascontiguousarray(dvB.T),
                rec=rec_c, node_pos=node_pos,
            )
        )

    return xlo, xhi, geom, per_core


def _wrap_idx(idx_flat):
    """int16 stream -> dma_gather layout [128, n/16]: value i at
    [i % 16, i // 16], replicated across the 8 groups of 16 partitions."""
    a = idx_flat.reshape(-1, 16).T
    return np.tile(a, (8, 1)).copy()


def _build_program(geom):
    SA, SB = geom["SA"], geom["SB"]
    NVA, NVB = geom["NVA"], geom["NVB"]
    MAXV = geom["MAXV"]
    SBp = max(SB, 1)
    NVBp = max(NVB, 1)

    nc = bacc.Bacc(
        "TRN2", target_bir_lowering=False, debug=False, num_swdge_queues=NQ
    )

    t_xlo = nc.dram_tensor("xlo", [SPLIT + 1, ROW], BF16, kind="ExternalInput")
    t_xhi = nc.dram_tensor(
        "xhi", [N_NODES - SPLIT + 1, ROW], BF16, kind="ExternalInput"
    )
    t_wt = nc.dram_tensor("wt", [D, D], BF16, kind="ExternalInput")
    t_b = nc.dram_tensor("bias", [D, 1], F32, kind="ExternalInput")
    t_ia = nc.dram_tensor("idxa", [128, SA * 8], I16, kind="ExternalInput")
    t_da = nc.dram_tensor("dsta", [128, NVA], F32, kind="ExternalInput")
    t_ib = nc.dram_tensor("idxb", [128, SBp * 8], I16, kind="ExternalInput")
    t_db = nc.dram_tensor("dstb", [128, NVBp], F32, kind="ExternalInput")
    t_iota = nc.dram_tensor(
        "iota", [128, GRP * MAXV * WIN], F32, kind="ExternalInput"
    )
    t_rec = nc.dram_tensor("rec", [D, NPC], F32, kind="ExternalInput")
    t_out = nc.dram_tensor("out", [D, NPC], F32, kind="ExternalOutput")

    callsA = [(p, min(CHUNK, SA - p)) for p in range(0, SA, CHUNK)]
    callsB = [(p, min(CHUNK, SB - p)) for p in range(0, SB, CHUNK)]

    with tile.TileContext(nc) as tc:
        with (
            tc.tile_pool(name="const", bufs=1) as cpool,
            tc.tile_pool(name="idx", bufs=1) as ipool,
            tc.tile_pool(name="msgsa", bufs=7) as mpa,
            tc.tile_pool(name="msgsb", bufs=5) as mpb,
            tc.tile_pool(name="oha", bufs=4) as opa,
            tc.tile_pool(name="ohb", bufs=4) as opb,
            tc.tile_pool(name="psacc", bufs=6, space="PSUM") as ps_acc,
            tc.tile_pool(name="psz", bufs=2, space="PSUM") as ps_z,
        ):
            # idx-stream loads go first (the first gathers wait on them);
            # constants that are needed later in the pipeline load after.
            ia_sb = ipool.tile([128, SA * 8], I16)
            ib_sb = ipool.tile([128, SBp * 8], I16)
            c0 = min(CHUNK * 8 * 4, SA * 8)
            nc.sync.dma_start(out=ia_sb[:, :c0], in_=t_ia[:, :c0])
            c1 = min(CHUNK * 8 * 2, SBp * 8)
            nc.sync.dma_start(out=ib_sb[:, :c1], in_=t_ib[:, :c1])
            da_sb = ipool.tile([128, NVA], F32)
            nc.sync.dma_start(out=da_sb[:], in_=t_da[:])
            db_sb = ipool.tile([128, NVBp], F32)
            nc.sync.dma_start(out=db_sb[:], in_=t_db[:])
            iota_f = cpool.tile([128, GRP * MAXV * WIN], F32)
            nc.sync.dma_start(out=iota_f[:], in_=t_iota[:])
            if c0 < SA * 8:
                nc.sync.dma_start(out=ia_sb[:, c0:], in_=t_ia[:, c0:])
            if c1 < SBp * 8:
                nc.sync.dma_start(out=ib_sb[:, c1:], in_=t_ib[:, c1:])
            rec_sb = cpool.tile([D, NPC], F32)
            nc.sync.dma_start(out=rec_sb[:], in_=t_rec[:])
            wt_sb = cpool.tile([D, D], BF16)
            nc.sync.dma_start(out=wt_sb[:], in_=t_wt[:])
            b_sb = cpool.tile([D, 1], F32)
            nc.sync.dma_start(out=b_sb[:], in_=t_b[:])

            ht_sb = cpool.tile([D, NPC], BF16)
            out_sb = cpool.tile([D, NPC], F32)

            chunk_tiles = {0: [], 1: []}
            call_no = [0]
            dma_sems = [
                nc.alloc_semaphore(f"gather_dma_q{q}") for q in range(NQ)
            ]
            prep_sems = [
                nc.alloc_semaphore(f"gather_prep_q{q}") for q in range(NQ)
            ]
            q_calls = [0] * NQ

            def emit_chunk(st, k):
                if st == 0:
                    pos, nsub = callsA[k]
                    mp, tsrc, isb = mpa, t_xlo, ia_sb
                else:
                    pos, nsub = callsB[k]
                    mp, tsrc, isb = mpb, t_xhi, ib_sb
                msgs = mp.tile([128, CHUNK, ROW], BF16)
                nidx = nsub * 128
                q = call_no[0] % NQ
                prep = nc.gpsimd.dma_gather(
                    msgs[:, :nsub, :],
                    tsrc[:],
                    isb[:, pos * 8 : pos * 8 + nsub * 8],
                    nidx,
                    nidx,
                    ROW,
                    single_packet=True,
                    prepare_only=PREP,
                    sem=dma_sems[q] if PREP else None,
                    queue_num=q,
                )
                q_calls[q] += 1
                if PREP:
                    # Tile's scheduler may float the trigger before its prep;
                    # gate it on the prep's gen-completion sem (the engine
                    # wait queue lets a parked trigger yield to later
                    # instructions, so this cannot deadlock).  count=1 fires
                    # the oldest untriggered ring entry (FIFO), which
                    # together with the >=k gate is always safe.
                    prep.then_inc(prep_sems[q], 1)
                    trig = nc.gpsimd.trigger_dma(count=1, queue_num=q)
                    trig._wait_ge(prep_sems[q], q_calls[q])
                call_no[0] += 1
                # Tile's pass-2 does not generate consumer RAW waits for
                # gen_mode=1 gathers (verified: no wait on gather_dma_q* in
                # the emitted program), so the first consumer must wait for
                # the DMA-completion sem explicitly.
                chunk_tiles[st].append((msgs, q, 16 * q_calls[q]))

            cursor = [0, 0]
            waited = set()

            def tiles_for(st, s):
                k = s // CHUNK
                while cursor[st] <= k:
                    emit_chunk(st, cursor[st])
                    cursor[st] += 1
                msgs, q, target = chunk_tiles[st][k]
                wait = None
                if PREP and (st, k) not in waited:
                    waited.add((st, k))
                    wait = (dma_sems[q], target)
                return msgs, s % CHUNK, wait

            vs0A, vs1A = geom["vs0A"], geom["vs1A"]
            vs0B, vs1B = geom["vs0B"], geom["vs1B"]
            pvA, pvB = geom["pvA"], geom["pvB"]

            def onehot(op_pool, dsb, c0, nv):
                # one DVE op builds the one-hots for a whole position group
                oh = op_pool.tile([128, GRP * MAXV * WIN], BF16)
                dst_b = (
                    dsb[:, c0 : c0 + nv]
                    .unsqueeze(2)
                    .to_broadcast([128, nv, WIN])
                )
                nc.vector.tensor_tensor(
                    out=oh[:, : nv * WIN].rearrange("p (s w) -> p s w", w=WIN),
                    in0=iota_f[:, : nv * WIN].rearrange(
                        "p (s w) -> p s w", w=WIN
                    ),
                    in1=dst_b,
                    op=mybir.AluOpType.is_equal,
                )
                return oh

            ohA = ohB = ps_g = None
            for p in range(N_WIN):
                if p % GRP == 0:
                    p0 = p
                    pend = min(p + GRP, N_WIN)
                    ohA = onehot(
                        opa, da_sb, int(pvA[p0]), int(pvA[pend] - pvA[p0])
                    )
                    nvtB = int(pvB[pend] - pvB[p0])
                    ohB = onehot(opb, db_sb, int(pvB[p0]), nvtB) if nvtB else None
                    ps_g = ps_acc.tile([D, GRP * WIN], F32)
                nA = int(vs1A[p] - vs0A[p])
                nB = int(vs1B[p] - vs0B[p])
                oA = int(pvA[p] - pvA[p0])
                oB = int(pvB[p] - pvB[p0])
                g = p - p0
                ps = ps_g[:, g * WIN : (g + 1) * WIN]
                tot = nA + nB
                j = 0
                for s in range(int(vs0A[p]), int(vs1A[p])):
                    msgs, col, wait = tiles_for(0, s)
                    jj = oA + s - int(vs0A[p])
                    mm = nc.tensor.matmul(
                        out=ps,
                        lhsT=msgs[:, col, :D],
                        rhs=ohA[:, jj * WIN : (jj + 1) * WIN],
                        start=(j == 0),
                        stop=(j == tot - 1),
                    )
                    if wait is not None:
                        mm._wait_ge(wait[0], wait[1])
                    j += 1
                for s in range(int(vs0B[p]), int(vs1B[p])):
                    msgs, col, wait = tiles_for(1, s)
                    jj = oB + s - int(vs0B[p])
                    mm = nc.tensor.matmul(
                        out=ps,
                        lhsT=msgs[:, col, :D],
                        rhs=ohB[:, jj * WIN : (jj + 1) * WIN],
                        start=(j == 0),
                        stop=(j == tot - 1),
                    )
                    if wait is not None:
                        mm._wait_ge(wait[0], wait[1])
                    j += 1
                if p == min(p0 + GRP, N_WIN) - 1:
                    gc = (p - p0 + 1) * WIN
                    nc.vector.tensor_tensor(
                        out=ht_sb[:, p0 * WIN : p0 * WIN + gc],
                        in0=ps_g[:, :gc],
                        in1=rec_sb[:, p0 * WIN : p0 * WIN + gc],
                        op=mybir.AluOpType.mult,
                    )
                t0 = (p // (ZCOL // WIN)) * ZCOL
                done = (p + 1) * WIN
                if done - t0 == ZCOL or p == N_WIN - 1:
                    zc = done - t0
                    z = ps_z.tile([D, ZCOL], F32)
                    nc.tensor.matmul(
                        out=z[:, :zc],
                        lhsT=wt_sb[:],
                        rhs=ht_sb[:, t0 : t0 + zc],
                        start=True,
                        stop=True,
                    )
                    nc.vector.tensor_scalar_add(
                        out_sb[:, t0 : t0 + zc], z[:, :zc], b_sb[:]
                    )
                    nc.sync.dma_start(
                        out=t_out[:, t0 : t0 + zc],
                        in_=out_sb[:, t0 : t0 + zc],
                    )

    nc.compile()
    return nc


def kernel(x, src, dst, W, b):
    x = np.asarray(x, dtype=np.float32)
    W = np.asarray(W, dtype=np.float32)
    b = np.asarray(b, dtype=np.float32)

    xlo, xhi, geom, per_core = _prep(x, src, dst)
    nc = _build_program(geom)

    wt = np.ascontiguousarray(W.T).astype(BF)
    bcol = np.ascontiguousarray(b.reshape(D, 1))
    iota_arr = np.tile(
        np.arange(WIN, dtype=np.float32)[None, :], (128, GRP * geom["MAXV"])
    ).copy()

    in_maps = []
    for c in range(N_CORES):
        pc = per_core[c]
        in_maps.append(
            {
                "xlo": xlo,
                "xhi": xhi,
                "wt": wt,
                "bias": bcol,
                "idxa": _wrap_idx(pc["iA"]),
                "dsta": pc["dvA"],
                "idxb": _wrap_idx(pc["iB"]),
                "dstb": pc["dvB"],
                "iota": iota_arr,
                "rec": pc["rec"],
            }
        )

    res = run_bass_kernel_spmd(nc, in_maps, list(range(N_CORES)))
    LAST["results"] = res
    LAST["exec_time_ns"] = res.exec_time_ns

    out = np.zeros((N_NODES, D), dtype=np.float32)
    for c in range(N_CORES):
        cols = res.results[c]["out"]  # [D, NPC]
        node_pos = per_core[c]["node_pos"]
        valid = node_pos >= 0
        out[node_pos[valid]] = cols[:, valid].T
    return np.ascontiguousarray(out)
